# revision 1
# baseline (speedup 1.0000x reference)
"""ChebNet (K=2, two ChebConv layers + log_softmax) on 8 Trainium2 NeuronCores.

Gather-free design (the graph is known on the host, which also does the
sharding): nodes are dealt into 32-dest blocks balanced by in-degree (snake
deal) across 8 cores x 416 blocks. For each layer the host materializes the
edge-source feature rows in "slot" order (XS) plus per-chunk selector
matrices SEL (Laplacian-weight one-hots over each 32-dest window). The
device then computes, per 512-dest supertile:

    XS_agg[Fin x 512] = sum_chunks XS_chunk.T @ SEL_chunk   (PE, PSUM accum)
    pre [Fout x 512]  = W1.T @ XS_agg + W0.T @ XT_own[:, st]
    layer 1: h.T = relu(pre + b1) -> transpose -> h rows
    layer 2: o.T = pre + b2 -> transpose -> log_softmax -> out rows

Layer 1 and layer 2 are two SPMD launches; the host re-orders layer-1's h
into layer-2 slot order between them (halo exchange through the host).
"""

import contextlib

import numpy as np
import jax
from jax.sharding import Mesh, PartitionSpec
from jax.experimental.shard_map import shard_map

import concourse.bass as bass
import concourse.mybir as mybir
import concourse.tile as tile
from concourse import bacc
from concourse.masks import make_identity
from concourse.bass2jax import (
    _bass_exec_p,
    install_neuronx_cc_hook,
    partition_id_tensor,
)

F32 = mybir.dt.float32

# problem constants (nn_ChebNet_15530601743030)
N = 100000
F_IN = 50
HID = 32
NCLS = 40
CORES = 8

P = 128
DBLOCK = 32            # dests per selector window
CPD = 4                # chunks per dblock (cap = CPD*128 slots per dblock)
ST_DB = 16             # dblocks per supertile
ST_W = DBLOCK * ST_DB  # 512 dest slots per supertile
DPC = 13312            # dest slots per core (26 supertiles)


# ---------------------------------------------------------------------------
# host-side schedule / data construction
# ---------------------------------------------------------------------------

def build_sigma(deg, n_cores, dpc):
    """Deal dests (by degree, snake order) into n_cores*dpc/DBLOCK dblocks."""
    n = deg.shape[0]
    ndb = n_cores * dpc // DBLOCK
    order = np.argsort(-deg, kind="stable")
    db_of = np.empty(n, np.int64)
    pos_in_db = np.empty(n, np.int64)
    for s in range(0, (n + ndb - 1) // ndb):
        chunk = order[s * ndb : (s + 1) * ndb]
        ids = np.arange(chunk.shape[0])
        tgt = ids if (s % 2 == 0) else (ndb - 1 - ids)
        db_of[chunk] = tgt
        pos_in_db[chunk] = s
    core_of = db_of % n_cores
    local_db = db_of // n_cores
    slot_of = local_db * DBLOCK + pos_in_db
    assert pos_in_db.max() < DBLOCK
    return core_of.astype(np.int64), slot_of.astype(np.int64)


def build_slot_layout(erow_core, erow_slot, ecol, ew, dpc, n_cores):
    """Per-core slot layout: xs_idx (source row per slot) and SEL matrix."""
    ndb_local = dpc // DBLOCK
    slots_per_db = CPD * P
    tot_slots = ndb_local * slots_per_db
    ch_total = ndb_local * CPD
    per_core = []
    for c in range(n_cores):
        m = erow_core == c
        slot = erow_slot[m]
        col = ecol[m]
        w = ew[m]
        db = slot // DBLOCK
        dloc = slot % DBLOCK
        ordi = np.argsort(db, kind="stable")
        db, dloc, col, w = db[ordi], dloc[ordi], col[ordi], w[ordi]
        counts = np.bincount(db, minlength=ndb_local)
        if counts.max() > slots_per_db:
            raise RuntimeError(f"dblock overflow: {counts.max()} > {slots_per_db}")
        starts = np.zeros(ndb_local + 1, np.int64)
        np.cumsum(counts, out=starts[1:])
        within = np.arange(db.shape[0]) - starts[db]
        gslot = db * slots_per_db + within
        xs_idx = np.zeros(tot_slots, np.int64)
        selv = np.zeros(tot_slots, np.float32)
        seld = np.zeros(tot_slots, np.int64)
        xs_idx[gslot] = col
        selv[gslot] = w
        seld[gslot] = dloc
        sel = np.zeros((P, ch_total * DBLOCK), np.float32)
        s = np.arange(tot_slots)
        sel[s % P, (s // P) * DBLOCK + seld] = selv
        per_core.append({"xs_idx": xs_idx, "sel": sel})
    return per_core


# ---------------------------------------------------------------------------
# device kernel (one ChebConv layer, SPMD over 8 cores)
# ---------------------------------------------------------------------------

def build_layer_kernel(fin, fout, dpc, layer, n_loop=1):
    nst = dpc // ST_W
    ndb_local = dpc // DBLOCK
    ch_total = ndb_local * CPD
    tot_slots = ndb_local * CPD * P
    g_ch = ST_DB * CPD  # 64 chunks per supertile

    nc = bacc.Bacc(None, target_bir_lowering=False)
    xs_d = nc.dram_tensor("xs", [P, ch_total * fin], F32, kind="ExternalInput")
    sel_d = nc.dram_tensor("sel", [P, ch_total * DBLOCK], F32, kind="ExternalInput")
    xtown_d = nc.dram_tensor("xtown", [fin, dpc], F32, kind="ExternalInput")
    w0_d = nc.dram_tensor("w0", [fin, fout], F32, kind="ExternalInput")
    w1_d = nc.dram_tensor("w1", [fin, fout], F32, kind="ExternalInput")
    b_d = nc.dram_tensor("b", [fout, 1], F32, kind="ExternalInput")
    out_d = nc.dram_tensor("out", [dpc, fout], F32, kind="ExternalOutput")

    with tile.TileContext(nc) as tc:
        loop_cm = tc.For_i(0, n_loop, 1) if n_loop > 1 else contextlib.nullcontext()
        with loop_cm:
            with (
                tc.tile_pool(name="const", bufs=1) as constp,
                tc.tile_pool(name="xsp", bufs=4) as xsp,
                tc.tile_pool(name="selp", bufs=4) as selp,
                tc.tile_pool(name="aggp", bufs=4) as aggp,
                tc.tile_pool(name="stgp", bufs=4) as stgp,
                tc.tile_pool(name="psx", bufs=3, space="PSUM") as psx,
                tc.tile_pool(name="psh", bufs=3, space="PSUM") as psh,
                tc.tile_pool(name="pse", bufs=2, space="PSUM") as pse,
            ):
                w0t = constp.tile([fin, fout], F32)
                nc.sync.dma_start(w0t[:], w0_d[:])
                w1t = constp.tile([fin, fout], F32)
                nc.sync.dma_start(w1t[:], w1_d[:])
                bt = constp.tile([fout, 1], F32)
                nc.sync.dma_start(bt[:], b_d[:])
                xot = constp.tile([fin, dpc], F32)
                nc.sync.dma_start(xot[:], xtown_d[:])
                ident = constp.tile([P, P], F32)
                make_identity(nc, ident[:])

                for st in range(nst):
                    xst = xsp.tile([P, g_ch, fin], F32, tag="xs")
                    nc.sync.dma_start(
                        xst[:],
                        xs_d[:, st * g_ch * fin : (st + 1) * g_ch * fin]
                        .rearrange("p (j f) -> p j f", f=fin),
                    )
                    selt = selp.tile([P, g_ch * DBLOCK], F32, tag="sel")
                    nc.scalar.dma_start(
                        selt[:],
                        sel_d[:, st * g_ch * DBLOCK : (st + 1) * g_ch * DBLOCK],
                    )
                    pxs = psx.tile([fin, ST_W], F32, tag="pxs")
                    for db in range(ST_DB):
                        for j in range(CPD):
                            ch = db * CPD + j
                            nc.tensor.matmul(
                                pxs[:, db * DBLOCK : (db + 1) * DBLOCK],
                                lhsT=xst[:, ch, :],
                                rhs=selt[:, ch * DBLOCK : (ch + 1) * DBLOCK],
                                start=(j == 0),
                                stop=(j == CPD - 1),
                            )
                    agg = aggp.tile([fin, ST_W], F32, tag="agg")
                    nc.scalar.activation(
                        agg[:], pxs[:], mybir.ActivationFunctionType.Copy
                    )
                    ph = psh.tile([fout, ST_W], F32, tag="ph")
                    nc.tensor.matmul(
                        ph[:], lhsT=w1t[:], rhs=agg[:], start=True, stop=False
                    )
                    nc.tensor.matmul(
                        ph[:],
                        lhsT=w0t[:],
                        rhs=xot[:, st * ST_W : (st + 1) * ST_W],
                        start=False,
                        stop=True,
                    )
                    ot = aggp.tile([fout, ST_W], F32, tag="ot")
                    if layer == 1:
                        nc.scalar.activation(
                            ot[:], ph[:], mybir.ActivationFunctionType.Relu,
                            bias=bt[:],
                        )
                    else:
                        nc.vector.tensor_tensor(
                            ot[:], ph[:], bt[:].to_broadcast([fout, ST_W]),
                            op=mybir.AluOpType.add,
                        )
                    pt = pse.tile([P, 4 * fout], F32, tag="pt")
                    for q in range(4):
                        nc.tensor.transpose(
                            pt[:, q * fout : (q + 1) * fout],
                            ot[:, q * P : (q + 1) * P],
                            ident[:fout, :fout],
                        )
                    stg = stgp.tile([P, 4 * fout], F32, tag="stg")
                    if layer == 1:
                        nc.vector.tensor_copy(stg[:], pt[:])
                    else:
                        pt3 = pt[:].rearrange("p (q f) -> p q f", f=fout)
                        mx = stgp.tile([P, 4, 1], F32, tag="mx")
                        nc.vector.tensor_reduce(
                            mx[:], pt3, op=mybir.AluOpType.max,
                            axis=mybir.AxisListType.X,
                        )
                        tsub = stgp.tile([P, 4, fout], F32, tag="tsub")
                        nc.vector.tensor_tensor(
                            tsub[:], pt3, mx[:].to_broadcast([P, 4, fout]),
                            op=mybir.AluOpType.subtract,
                        )
                        ex = stgp.tile([P, 4, fout], F32, tag="ex")
                        nc.scalar.activation(
                            ex[:].rearrange("p q f -> p (q f)"),
                            tsub[:].rearrange("p q f -> p (q f)"),
                            mybir.ActivationFunctionType.Exp,
                        )
                        sm = stgp.tile([P, 4, 1], F32, tag="sm")
                        nc.vector.tensor_reduce(
                            sm[:], ex[:], op=mybir.AluOpType.add,
                            axis=mybir.AxisListType.X,
                        )
                        ls = stgp.tile([P, 4, 1], F32, tag="ls")
                        nc.scalar.activation(
                            ls[:], sm[:], mybir.ActivationFunctionType.Ln
                        )
                        nc.vector.tensor_tensor(
                            stg[:].rearrange("p (q f) -> p q f", f=fout),
                            tsub[:], ls[:].to_broadcast([P, 4, fout]),
                            op=mybir.AluOpType.subtract,
                        )
                    nc.scalar.dma_start(
                        out_d[:].rearrange("(s q p) f -> s p q f", q=4, p=P)[st],
                        stg[:].rearrange("p (q f) -> p q f", f=fout),
                    )
    nc.finalize()
    return nc


# ---------------------------------------------------------------------------
# PJRT SPMD runner (jit once, device-resident inputs)
# ---------------------------------------------------------------------------

class SpmdRunner:
    def __init__(self, nc, n_cores):
        install_neuronx_cc_hook()
        assert nc.is_finalized()
        self.nc = nc
        self.n_cores = n_cores
        partition_name = (
            nc.partition_id_tensor.name if nc.partition_id_tensor else None
        )
        in_names, out_names, out_avals, zero_outs = [], [], [], []
        for alloc in nc.m.functions[0].allocations:
            if not isinstance(alloc, mybir.MemoryLocationSet):
                continue
            name = alloc.memorylocations[0].name
            if alloc.kind == "ExternalInput":
                if name != partition_name:
                    in_names.append(name)
            elif alloc.kind == "ExternalOutput":
                out_names.append(name)
                shape = tuple(alloc.tensor_shape)
                dtype = mybir.dt.np(alloc.dtype)
                out_avals.append(jax.core.ShapedArray(shape, dtype))
                zero_outs.append(np.zeros(shape, dtype))
        self.in_names = in_names
        self.out_names = out_names
        self.out_avals = out_avals
        self.zero_outs = zero_outs
        n_params = len(in_names)
        n_outs = len(out_avals)
        all_in_names = list(in_names) + list(out_names)
        if partition_name is not None:
            all_in_names.append(partition_name)

        def _body(*args):
            operands = list(args)
            if partition_name is not None:
                operands.append(partition_id_tensor())
            outs = _bass_exec_p.bind(
                *operands,
                out_avals=tuple(out_avals),
                in_names=tuple(all_in_names),
                out_names=tuple(out_names),
                lowering_input_output_aliases=(),
                sim_require_finite=True,
                sim_require_nnan=True,
                nc=nc,
            )
            return tuple(outs)

        devices = jax.devices()[:n_cores]
        assert len(devices) == n_cores
        self.mesh = Mesh(np.asarray(devices), ("core",))
        in_specs = (PartitionSpec("core"),) * (n_params + n_outs)
        out_specs = (PartitionSpec("core"),) * len(out_names)
        self.fn = jax.jit(
            shard_map(
                _body, mesh=self.mesh, in_specs=in_specs,
                out_specs=out_specs, check_rep=False,
            ),
            keep_unused=True,
        )
        self._dev_zeros = None
        self._staged = None

    def stage_inputs(self, in_maps):
        sharding = jax.sharding.NamedSharding(self.mesh, PartitionSpec("core"))
        concat = []
        for name in self.in_names:
            arrs = [np.asarray(m[name]) for m in in_maps]
            concat.append(jax.device_put(np.concatenate(arrs, axis=0), sharding))
        if self._dev_zeros is None:
            self._dev_zeros = [
                jax.device_put(
                    np.zeros((self.n_cores * z.shape[0], *z.shape[1:]), z.dtype),
                    sharding,
                )
                for z in self.zero_outs
            ]
        self._staged = concat

    def run_blocking(self):
        outs = self.fn(*self._staged, *self._dev_zeros)
        jax.block_until_ready(outs)
        return outs

    def fetch(self, outs):
        return [
            {
                name: np.asarray(outs[i]).reshape(
                    self.n_cores, *self.out_avals[i].shape
                )[c]
                for i, name in enumerate(self.out_names)
            }
            for c in range(self.n_cores)
        ]


_RUNNERS = {}


def _get_runner(fin, fout, dpc, layer, n_loop=1):
    key = (fin, fout, dpc, layer, n_loop)
    if key not in _RUNNERS:
        nc = build_layer_kernel(fin, fout, dpc, layer, n_loop=n_loop)
        _RUNNERS[key] = SpmdRunner(nc, CORES)
    return _RUNNERS[key]


# ---------------------------------------------------------------------------
# top-level entry
# ---------------------------------------------------------------------------

def _preprocess(edge_index):
    row = np.asarray(edge_index[0]).astype(np.int64)
    col = np.asarray(edge_index[1]).astype(np.int64)
    valid = row != col
    deg = np.bincount(row[valid], minlength=N).astype(np.float32)
    dis = np.where(
        deg > 0, 1.0 / np.sqrt(np.maximum(deg, 1.0), dtype=np.float32), 0.0
    ).astype(np.float32)
    w = (-dis[row] * dis[col]).astype(np.float32) * valid
    keep = w != 0
    er, ec, ew = row[keep], col[keep], w[keep].astype(np.float32)
    core_of, slot_of = build_sigma(deg.astype(np.float64), CORES, DPC)
    layout = build_slot_layout(core_of[er], slot_of[er], ec, ew, DPC, CORES)
    return core_of, slot_of, layout


def _run_layer(layer, fin, fout, src_rows, own_rows, layout, core_of, slot_of,
               W0, W1, b, n_loop=1):
    r = _get_runner(fin, fout, DPC, layer, n_loop)
    in_maps = []
    for c in range(CORES):
        xs_rows = src_rows[layout[c]["xs_idx"]].astype(np.float32)
        ch_total = xs_rows.shape[0] // P
        xs = np.ascontiguousarray(
            xs_rows.reshape(ch_total, P, fin).transpose(1, 0, 2).reshape(
                P, ch_total * fin
            )
        )
        xtown = np.zeros((fin, DPC), np.float32)
        mine = np.where(core_of == c)[0]
        xtown[:, slot_of[mine]] = own_rows[mine].T
        in_maps.append(
            {
                "xs": xs,
                "sel": layout[c]["sel"],
                "xtown": xtown,
                "w0": np.asarray(W0, np.float32),
                "w1": np.asarray(W1, np.float32),
                "b": np.asarray(b, np.float32).reshape(fout, 1),
            }
        )
    r.stage_inputs(in_maps)
    outs = r.fetch(r.run_blocking())
    full = np.zeros((N, fout), np.float32)
    for c in range(CORES):
        mine = np.where(core_of == c)[0]
        full[mine] = outs[c]["out"][slot_of[mine]]
    return full


def kernel(x, edge_index, W0_1, W1_1, b1, W0_2, W1_2, b2):
    x = np.asarray(x, dtype=np.float32)
    core_of, slot_of, layout = _preprocess(edge_index)
    h = _run_layer(
        1, F_IN, HID, x, x, layout, core_of, slot_of, W0_1, W1_1, b1
    )
    out = _run_layer(
        2, HID, NCLS, h, h, layout, core_of, slot_of, W0_2, W1_2, b2
    )
    return out

